# revision 20
# baseline (speedup 1.0000x reference)
"""Trainium2 Bass kernel for nn_PhysicsGuidedNuclearModel (self-contained).

Strategy: batch-data-parallel SPMD over 8 NeuronCores (4 of 32 rows each).
Activations are [feature-on-partition, (b,t)-on-free] fp32 SBUF tiles.

Mamba selective scan:
  h[t,d,n] = exp(A[d,n]*delta[t,n]) * h[t-1,d,n] + delta[t,n]*u[t,d]
  y[t,d]   = sum_n (C*B)[d,n] * h[t,d,n]          (B folded out of the scan)
Lanes are (d-sub 8, n 16) per d-block (16 blocks):
  - delta broadcast lane-wise via PE matmul with a 16x128 selection matrix
  - exp on ACT with per-partition scale = A[d,n]
  - u replicated per block via PE matmul with a 128x128 selection matrix
  - data1 = delta_rep * u_rep on DVE; scan via tensor_tensor_scan on
    GPSIMD/DVE; y accumulated on PE with block-diagonal (C*B) matrices.
All other layers: PE matmuls + ACT activations; LayerNorm over the partition
axis uses PE ones-matmuls for the moments and PE K=1 broadcasts to apply
per-column scale/offset. Eval-mode BatchNorms and mean-pool scales are folded
into per-channel scale/bias host-side; pure biases fold into matmul ones-rows.

The installed walrus build accepts at most ONE embedded sync wait per engine
instruction, so after Tile scheduling we hoist surplus waits onto standalone
EventSemaphore instructions placed immediately before the owner.
"""
import numpy as np
from contextlib import ExitStack

import concourse.bass as bass
import concourse.tile as tile
import concourse.mybir as mybir
from concourse.bass_utils import run_bass_kernel_spmd

F32 = mybir.dt.float32
AF = mybir.ActivationFunctionType
OP = mybir.AluOpType

B, T, P = 32, 512, 52
DIM, ED, DS, DC = 64, 128, 16, 4
NUM_CLASSES = 12
NCORES = 8
BL = B // NCORES          # 4 local batch rows
BT = BL * T               # 2048
NBLK = 16                 # ED / 8 d-blocks; lanes per block = 8*16 = 128
EPS = 1e-5

ABAR_BUFS = 2
DATA1_BUFS = 2
HT_BUFS = 2
TRACE = False      # set by test harness to capture an NTFF profile
LAST_RESULT = None  # BassKernelResults of the most recent kernel() call


# ----------------------------------------------------------------------------
# host-side constants
# ----------------------------------------------------------------------------

def _host_constants(params):
    p = params
    c = {}
    f32 = np.float32

    def put(name, arr):
        c[name] = np.ascontiguousarray(arr, dtype=f32)

    # tfe_lin1 with bias folded in as a ones-row
    put('W1T', np.vstack([np.asarray(p['tfe_lin1_w'], f32).T,
                          np.asarray(p['tfe_lin1_b'], f32)[None, :]]))   # [53,64]
    for mi, mkey in enumerate(['mamba1', 'mamba2']):
        m = p[mkey]
        put(f'WinTu{mi}', m['in_proj'][:ED].T)          # [64,128]
        put(f'WinTg{mi}', m['in_proj'][ED:].T)          # [64,128]
        for k in range(DC):
            put(f'Dk{mi}_{k}', np.diag(np.asarray(m['conv_w'], f32)[:, k]))
        put(f'Bm{mi}', m['Bm'])                          # [128,16]
        Pn = np.zeros((DS, ED), f32)
        for j in range(8):
            for n in range(DS):
                Pn[n, j * DS + n] = 1.0
        put(f'Pn{mi}', Pn)                               # [16,128]
        A = -np.exp(np.asarray(m['A'], f32))
        CB = np.asarray(m['Cm'], f32) * np.asarray(m['Bm'], f32)
        avec = np.zeros((ED, NBLK), f32)                 # [lane, blk]
        for blk in range(NBLK):
            Q = np.zeros((ED, ED), f32)
            W = np.zeros((ED, ED), f32)
            for j in range(8):
                d = blk * 8 + j
                for n in range(DS):
                    lane = j * DS + n
                    avec[lane, blk] = A[d, n]
                    Q[d, lane] = 1.0
                    W[lane, d] = CB[d, n]
            put(f'Q{mi}_{blk}', Q)
            put(f'WCB{mi}_{blk}', W)
        put(f'avec{mi}', avec)
        put(f'WoutT{mi}', m['out_proj'].T)               # [128,64]
    put('I64', np.eye(DIM, dtype=f32))
    put('ones64', np.ones((DIM, 1), f32))                # lhsT for column sums
    lng = np.stack([np.asarray(p['ln1_g'], f32), np.asarray(p['ln2_g'], f32)], 1)
    lnb = np.stack([np.asarray(p['ln1_b'], f32), np.asarray(p['ln2_b'], f32)], 1)
    for i, row in enumerate([lng[:, 0], lnb[:, 0], lng[:, 1], lnb[:, 1]]):
        put(f'lnrow{i}', row.reshape(1, DIM))
    put('W2T', p['tfe_lin2_w'].T)                        # [64,128]
    put('b2', p['tfe_lin2_b'].reshape(128, 1))
    # CNN (BN + conv-bias folded into scale/bias)
    s1 = np.asarray(p['bn1_g'], f32) / np.sqrt(np.asarray(p['bn1_v'], f32) + EPS)
    put('cnn1s', s1.reshape(32, 1))
    put('cnn1b', ((p['conv1_b'] - p['bn1_m']) * s1 + p['bn1_b']).reshape(32, 1))
    for k in range(3):
        put(f'W1k{k}', p['conv1_w'][:, :, k].T)          # [52,32]
    s2 = np.asarray(p['bn2_g'], f32) / np.sqrt(np.asarray(p['bn2_v'], f32) + EPS)
    put('cnn2s', s2.reshape(64, 1))
    put('cnn2b', ((p['conv2_b'] - p['bn2_m']) * s2 + p['bn2_b']).reshape(64, 1))
    for k in range(3):
        put(f'W2k{k}', p['conv2_w'][:, :, k].T)          # [32,64]
    # physics
    put('Wp1T', p['phys_w1'].T)                          # [5,16]
    put('bp1', p['phys_b1'].reshape(16, 1))
    put('Wp2T', p['phys_w2'].T)                          # [16,32]
    put('bp2', p['phys_b2'].reshape(32, 1))
    # fusion (mean-pool scales folded into fus_w1 columns; BNs into scale/bias)
    rowscale = np.concatenate([
        np.full(128, 1.0 / T, f32), np.full(64, 1.0 / 256.0, f32), np.ones(32, f32)])
    Wf1T = np.asarray(p['fus_w1'], f32).T * rowscale[:, None]   # [224,256]
    put('Wf1Ta', Wf1T[:128])                             # [128,256]
    put('Wf1Tb', Wf1T[128:])                             # [96,256]
    sf1 = np.asarray(p['fbn1_g'], f32) / np.sqrt(np.asarray(p['fbn1_v'], f32) + EPS)
    put('f1s', sf1.reshape(2, 128).T)
    f1b_full = ((p['fus_b1'] - p['fbn1_m']) * sf1 + p['fbn1_b'])
    put('f1b', np.asarray(f1b_full, f32).reshape(2, 128).T)
    Wf2T = np.asarray(p['fus_w2'], f32).T                # [256,128]
    put('Wf2Ta', Wf2T[:128])
    put('Wf2Tb', Wf2T[128:])
    sf2 = np.asarray(p['fbn2_g'], f32) / np.sqrt(np.asarray(p['fbn2_v'], f32) + EPS)
    put('f2s', sf2.reshape(128, 1))
    put('f2b', ((p['fus_b2'] - p['fbn2_m']) * sf2 + p['fbn2_b']).reshape(128, 1))
    put('Wc1T', p['cls_w1'].T)                           # [128,64]
    put('bc1', p['cls_b1'].reshape(64, 1))
    # classifier head bias folded via ones-row
    put('Wc2T', np.vstack([np.asarray(p['cls_w2'], f32).T,
                           np.asarray(p['cls_b2'], f32)[None, :]]))      # [65,12]
    put('xones', np.ones((1, BL, T + 2), f32))
    return c


def _const_specs():
    s = {}
    s['W1T'] = (P + 1, DIM)
    for mi in range(2):
        s[f'WinTu{mi}'] = (DIM, ED); s[f'WinTg{mi}'] = (DIM, ED)
        for k in range(DC):
            s[f'Dk{mi}_{k}'] = (ED, ED)
        s[f'Bm{mi}'] = (ED, DS); s[f'Pn{mi}'] = (DS, ED)
        for blk in range(NBLK):
            s[f'Q{mi}_{blk}'] = (ED, ED)
            s[f'WCB{mi}_{blk}'] = (ED, ED)
        s[f'avec{mi}'] = (ED, NBLK)
        s[f'WoutT{mi}'] = (ED, DIM)
    s['I64'] = (DIM, DIM); s['ones64'] = (DIM, 1)
    for i in range(4):
        s[f'lnrow{i}'] = (1, DIM)
    s['W2T'] = (DIM, 128); s['b2'] = (128, 1)
    s['cnn1s'] = (32, 1); s['cnn1b'] = (32, 1)
    for k in range(3):
        s[f'W1k{k}'] = (P, 32)
    s['cnn2s'] = (64, 1); s['cnn2b'] = (64, 1)
    for k in range(3):
        s[f'W2k{k}'] = (32, 64)
    s['Wp1T'] = (5, 16); s['bp1'] = (16, 1)
    s['Wp2T'] = (16, 32); s['bp2'] = (32, 1)
    s['Wf1Ta'] = (128, 256); s['Wf1Tb'] = (96, 256)
    s['f1s'] = (128, 2); s['f1b'] = (128, 2)
    s['Wf2Ta'] = (128, 128); s['Wf2Tb'] = (128, 128)
    s['f2s'] = (128, 1); s['f2b'] = (128, 1)
    s['Wc1T'] = (128, DIM); s['bc1'] = (DIM, 1)
    s['Wc2T'] = (DIM + 1, NUM_CLASSES)
    s['xones'] = (1, BL, T + 2)
    return s




def _pack_layout():
    """Column offsets for all 2-D consts packed into one [128, N] tensor."""
    specs = _const_specs()
    layout = {}
    off = 0
    for name, shape in specs.items():
        if name in ('xones',):
            continue
        p = shape[0]
        w = int(np.prod(shape[1:]))
        layout[name] = (p, off, w)
        off += w
    return layout, off


def _pack_consts(c):
    layout, total = _pack_layout()
    out = np.zeros((128, total), np.float32)
    for name, (p, off, w) in layout.items():
        out[0:p, off:off + w] = c[name].reshape(p, w)
    return out


# ----------------------------------------------------------------------------
# post-pass: hoist surplus sync waits (walrus allows 1 per engine instruction)
# ----------------------------------------------------------------------------

def _split_sync_waits(nc):
    for fn in nc.m.functions:
        for block in fn.blocks:
            out = []
            changed = False
            for inst in block.instructions:
                si = inst.sync_info
                if (si is not None and len(si.on_wait) > 1
                        and not isinstance(inst, mybir.InstEventSemaphore)):
                    waits = list(si.on_wait)
                    for i, w in enumerate(waits[:-1]):
                        ev = mybir.InstEventSemaphore(
                            name=f"{inst.name}_hw{i}",
                            engine=inst.engine,
                            sync_info=mybir.SyncInfo(on_wait=[w], on_update=[]),
                        )
                        out.append(ev)
                    si.on_wait = waits[-1:]
                    changed = True
                out.append(inst)
            if changed:
                block.instructions = out
    return nc


# ----------------------------------------------------------------------------
# device program
# ----------------------------------------------------------------------------

def _emit_mamba(nc, tc, cst, pools, tin, mi, dbg=None):
    """tin: SBUF tile [64, BL, T]. Returns g = gelu(mamba(tin)) [64, BL, T]."""
    big, loop, tmp = pools
    # --- in_proj: gate then u ---
    with tc.tile_pool(name=f"m{mi}ps_in", bufs=1, space="PSUM") as psp:
        ps_g = psp.tile([ED, BL, T], F32, tag="ps_g")
        for b in range(BL):
            nc.tensor.matmul(ps_g[:, b, :], cst[f'WinTg{mi}'], tin[:, b, :],
                             start=True, stop=True)
        sg = tmp.tile([ED, BL, T], F32, name="sg", tag="seq8")
        nc.scalar.activation(sg, ps_g, AF.Sigmoid)

        ps_u = psp.tile([ED, BL, T], F32, tag="ps_u")
        for b in range(BL):
            nc.tensor.matmul(ps_u[:, b, :], cst[f'WinTu{mi}'], tin[:, b, :],
                             start=True, stop=True)
        u_gated = tmp.tile([ED, BL, T + 3], F32, name="u_gated", tag="mam8")
        nc.vector.memset(u_gated[:, :, 0:3], 0.0)
        nc.vector.tensor_tensor(u_gated[:, :, 3:], ps_u, sg, op=OP.mult)

    # --- depthwise causal conv (4 diag matmuls) + silu ---
    with tc.tile_pool(name=f"m{mi}ps_cv", bufs=1, space="PSUM") as psp:
        ps_c = psp.tile([ED, BL, T], F32, tag="ps_c")
        for k in range(DC):
            for b in range(BL):
                nc.tensor.matmul(ps_c[:, b, :], cst[f'Dk{mi}_{k}'],
                                 u_gated[:, b, k:k + T],
                                 start=(k == 0), stop=(k == DC - 1))
        u_conv = big.tile([ED, BL, T], F32, tag="u_conv")
        nc.scalar.activation(u_conv, ps_c, AF.Silu)
    if dbg:
        dbg(f'dbg_uconv{mi}', u_conv)

    # --- delta + lane-broadcast ---
    with tc.tile_pool(name=f"m{mi}ps_d", bufs=1, space="PSUM") as psp:
        ps_dp = psp.tile([DS, BL, T], F32, tag="ps_dp")
        for b in range(BL):
            nc.tensor.matmul(ps_dp[:, b, :], cst[f'Bm{mi}'], u_conv[:, b, :],
                             start=True, stop=True)
        delta = tmp.tile([DS, BL, T], F32, name="delta", tag="seq8")
        nc.scalar.activation(delta, ps_dp, AF.Sigmoid)
        ps_dr = psp.tile([ED, BL, T], F32, tag="ps_dr")
        for b in range(BL):
            nc.tensor.matmul(ps_dr[:, b, :], cst[f'Pn{mi}'], delta[:, b, :],
                             start=True, stop=True)
        d_rep = big.tile([ED, BL, T], F32, tag="d_rep")
        nc.scalar.activation(d_rep, ps_dr, AF.Copy)
    if dbg:
        dbg(f'dbg_drep{mi}', d_rep)

    # --- selective scan over 16 d-blocks ---
    with tc.tile_pool(name=f"m{mi}ps_y", bufs=4, space="PSUM") as psy, \
         tc.tile_pool(name=f"m{mi}ps_ur", bufs=2, space="PSUM") as psur:
        y_ps = [psy.tile([ED, T], F32, name=f"y_ps{_b}", tag="y_ps") for _b in range(BL)]
        prev_h = None
        prev_blk = None
        for blk in range(NBLK):
            abar = loop.tile([ED, BL, T], F32, tag="abar", bufs=ABAR_BUFS)
            nc.scalar.activation(abar, d_rep, AF.Exp,
                                 scale=cst[f'avec{mi}'][:, blk:blk + 1])
            nc.gpsimd.memset(abar[:, :, 0:1], 0.0)
            data1 = loop.tile([ED, BL, T], F32, tag="data1", bufs=DATA1_BUFS)
            for half in range(2):
                ps_ur = psur.tile([ED, 2, T], F32, name="ps_ur", tag="ps_ur")
                for bb in range(2):
                    b = half * 2 + bb
                    nc.tensor.matmul(ps_ur[:, bb, :], cst[f'Q{mi}_{blk}'],
                                     u_conv[:, b, :], start=True, stop=True)
                nc.vector.tensor_tensor(
                    data1[:, half * 2:(half + 1) * 2, :],
                    d_rep[:, half * 2:(half + 1) * 2, :], ps_ur, op=OP.mult)
            h_t = loop.tile([ED, BL, T], F32, tag="h_t", bufs=HT_BUFS)
            nc.vector.tensor_tensor_scan(
                h_t.rearrange("p b t -> p (b t)"),
                abar.rearrange("p b t -> p (b t)"),
                data1.rearrange("p b t -> p (b t)"), 0.0,
                op0=OP.mult, op1=OP.add)
            # software pipeline: contract the PREVIOUS block's h so the PE
            # stream never stalls on this block's scan before issuing the
            # next block's u_rep matmuls
            if prev_h is not None:
                for b in range(BL):
                    nc.tensor.matmul(y_ps[b], cst[f'WCB{mi}_{prev_blk}'],
                                     prev_h[:, b, :],
                                     start=(prev_blk == 0), stop=False)
            prev_h, prev_blk = h_t, blk
        for b in range(BL):
            nc.tensor.matmul(y_ps[b], cst[f'WCB{mi}_{prev_blk}'],
                             prev_h[:, b, :], start=False, stop=True)
        y_sb = tmp.tile([ED, BL, T], F32, name="y_sb", tag="mam8")
        for b in range(BL):
            nc.scalar.activation(y_sb[:, b, :], y_ps[b], AF.Copy)
    if dbg:
        dbg(f'dbg_ysb{mi}', y_sb)

    # --- out_proj + residual + gelu ---
    with tc.tile_pool(name=f"m{mi}ps_o", bufs=1, space="PSUM") as psp:
        ps_t1 = psp.tile([DIM, BL, T], F32, tag="ps_t1")
        for b in range(BL):
            nc.tensor.matmul(ps_t1[:, b, :], cst[f'WoutT{mi}'], y_sb[:, b, :],
                             start=True, stop=False)
        for b in range(BL):
            nc.tensor.matmul(ps_t1[:, b, :], cst['I64'], tin[:, b, :],
                             start=False, stop=True)
        g = big.tile([DIM, BL, T], F32, tag="g_out")
        nc.scalar.activation(g, ps_t1, AF.Gelu)
    if dbg:
        dbg(f'dbg_g{mi}', g)
    return g


def _emit_layernorm(nc, tc, cst, pools, g, li):
    """LN over the partition axis (64) per column. g: [64, BL, T] (gelu'd)."""
    big, loop, tmp = pools
    sq = tmp.tile([DIM, BL, T], F32, name="ln_sq", tag="seq8")
    nc.scalar.activation(sq, g, AF.Square)
    t_out = big.tile([DIM, BL, T], F32, tag="t_seq")
    with tc.tile_pool(name=f"ln{li}ps", bufs=2, space="PSUM") as psp, \
         tc.tile_pool(name=f"ln{li}ps2", bufs=2, space="PSUM") as psp2:
        for b in range(BL):
            ps_s1 = psp.tile([1, T], F32, name="ps_s1", tag="ps_s1")
            ps_s2 = psp.tile([1, T], F32, name="ps_s2", tag="ps_s2")
            nc.tensor.matmul(ps_s1, cst['ones64'], g[:, b, :], start=True, stop=True)
            nc.tensor.matmul(ps_s2, cst['ones64'], sq[:, b, :], start=True, stop=True)
            mu2 = loop.tile([1, T], F32, name="ln_mu2", tag="ln_row", bufs=3)
            nc.scalar.activation(mu2, ps_s1, AF.Square, scale=1.0 / DIM)
            var = loop.tile([1, T], F32, name="ln_var", tag="ln_row", bufs=3)
            nc.vector.scalar_tensor_tensor(var, ps_s2, 1.0 / DIM, mu2,
                                           op0=OP.mult, op1=OP.subtract)
            lnv = loop.tile([1, T], F32, name="ln_lnv", tag="ln_row", bufs=3)
            nc.scalar.activation(lnv, var, AF.Ln, bias=cst['eps1'][0:1, 0:1])
            rstd = loop.tile([1, T], F32, name="ln_rstd", tag="ln_row", bufs=3)
            nc.scalar.activation(rstd, lnv, AF.Exp, scale=-0.5)
            crow = loop.tile([1, T], F32, name="ln_crow", tag="ln_row", bufs=3)
            nc.vector.scalar_tensor_tensor(crow, ps_s1, -1.0 / DIM, rstd,
                                           op0=OP.mult, op1=OP.mult)
            ps_sbc = psp2.tile([DIM, T], F32, name="ps_sbc", tag="ps_sbc")
            nc.tensor.matmul(ps_sbc, cst[f'lnrow{2 * li}'], rstd,
                             start=True, stop=True)
            ps_cbc = psp2.tile([DIM, T], F32, name="ps_cbc", tag="ps_cbc")
            nc.tensor.matmul(ps_cbc, cst[f'lnrow{2 * li}'], crow,
                             start=True, stop=False)
            nc.tensor.matmul(ps_cbc, cst[f'lnrow{2 * li + 1}'],
                             cst['onesrow'][0:1, :], start=False, stop=True)
            tmp_t = loop.tile([DIM, T], F32, tag="ln_tmp")
            nc.vector.tensor_tensor(tmp_t, g[:, b, :], ps_sbc, op=OP.mult)
            nc.vector.tensor_tensor(t_out[:, b, :], tmp_t, ps_cbc, op=OP.add)
    return t_out


def build_module(split_waits=True, debug=False):
    nc = bass.Bass("TRN2", target_bir_lowering=False, debug=False)
    consts = _const_specs()
    layout, total_cols = _pack_layout()
    dram = {}
    dram['x_loc'] = nc.dram_tensor("x_loc", [BL, T, P], F32, kind="ExternalInput")
    dram['phys_loc'] = nc.dram_tensor("phys_loc", [BL, 5], F32, kind="ExternalInput")
    dram['xones'] = nc.dram_tensor("xones", list(consts['xones']), F32,
                                   kind="ExternalInput")
    dram['cpack'] = nc.dram_tensor("cpack", [128, total_cols], F32,
                                   kind="ExternalInput")
    out_logits = nc.dram_tensor("logits", [BL, NUM_CLASSES], F32, kind="ExternalOutput")
    out_f = nc.dram_tensor("f", [BL, 128], F32, kind="ExternalOutput")
    dbg_dram = {}

    def _mk_dbg(nc_, tc_holder):
        def dbg(name, ap):
            dt = nc_.dram_tensor(name, list(ap.shape), F32, kind="ExternalOutput")
            dbg_dram[name] = dt
            nc_.sync.dma_start(dt.ap(), ap)
        return dbg

    with tile.TileContext(nc) as tc, ExitStack() as ctx:
        cpool = ctx.enter_context(tc.tile_pool(name="consts", bufs=1))
        big = ctx.enter_context(tc.tile_pool(name="big", bufs=1))
        loop = ctx.enter_context(tc.tile_pool(name="loop", bufs=2))
        tmp = ctx.enter_context(tc.tile_pool(name="tmp", bufs=1))
        pools = (big, loop, tmp)

        cpack = cpool.tile([128, total_cols], F32, tag="cpack")
        nc.sync.dma_start(cpack, dram['cpack'].ap())
        cst = {}
        for name, (pdim, off, w) in layout.items():
            cst[name] = cpack[0:pdim, off:off + w]
        eps1 = cpool.tile([1, 1], F32, tag="eps1")
        nc.vector.memset(eps1, EPS)
        cst['eps1'] = eps1
        onesrow = cpool.tile([1, T], F32, tag="onesrow")
        nc.vector.memset(onesrow, 1.0)
        cst['onesrow'] = onesrow

        # ---- x into [53, BL, T+2]: row 52 = ones (lin1 bias), 1 zero col/side ----
        x_pad = big.tile([P + 1, BL, T + 2], F32, tag="x_pad")
        nc.vector.memset(x_pad[:, :, 0:1], 0.0)
        nc.vector.memset(x_pad[:, :, T + 1:T + 2], 0.0)
        nc.sync.dma_start(x_pad[P:P + 1, :, :], dram['xones'].ap())
        xap = dram['x_loc'].ap()
        for b in range(BL):
            nc.sync.dma_start(x_pad[0:P, b, 1:T + 1], xap[b].rearrange("t p -> p t"))

        # ---- tfe_lin1 (bias via ones-row) ----
        with tc.tile_pool(name="ps_l1", bufs=1, space="PSUM") as psp:
            ps_t0 = psp.tile([DIM, BL, T], F32, tag="ps_t0")
            for b in range(BL):
                nc.tensor.matmul(ps_t0[:, b, :], cst['W1T'], x_pad[:, b, 1:T + 1],
                                 start=True, stop=True)
            t0 = big.tile([DIM, BL, T], F32, tag="t_seq")
            nc.scalar.activation(t0, ps_t0, AF.Copy)

        dbg = _mk_dbg(nc, None) if debug else None
        if dbg:
            dbg('dbg_t0', t0)
        # ---- mamba1 + LN1, mamba2 + LN2 ----
        g1 = _emit_mamba(nc, tc, cst, pools, t0, 0, dbg)
        t2 = _emit_layernorm(nc, tc, cst, pools, g1, 0)
        if dbg:
            dbg('dbg_t2', t2)
        g2 = _emit_mamba(nc, tc, cst, pools, t2, 1, dbg)
        t4 = _emit_layernorm(nc, tc, cst, pools, g2, 1)
        if dbg:
            dbg('dbg_t4', t4)

        # ---- tfe_lin2 + gelu + time mean ----
        f_inA = big.tile([128, BL], F32, tag="f_inA")
        with tc.tile_pool(name="ps_l2", bufs=1, space="PSUM") as psp:
            ps_tf = psp.tile([128, BL, T], F32, tag="ps_tf")
            for b in range(BL):
                nc.tensor.matmul(ps_tf[:, b, :], cst['W2T'], t4[:, b, :],
                                 start=True, stop=True)
            tf = tmp.tile([128, BL, T], F32, name="tf", tag="cnn_tmp")
            nc.scalar.activation(tf, ps_tf, AF.Gelu, bias=cst['b2'][:, 0:1])
        for b in range(BL):
            nc.vector.tensor_reduce(f_inA[:, b:b + 1], tf[:, b, :],
                                    axis=mybir.AxisListType.X, op=OP.add)

        # ---- CNN branch ----
        f_inB = big.tile([96, BL], F32, tag="f_inB")
        with tc.tile_pool(name="ps_cnn", bufs=1, space="PSUM") as psp:
            ps_c1 = psp.tile([32, BL, T], F32, tag="ps_c1")
            for k in range(3):
                for b in range(BL):
                    nc.tensor.matmul(ps_c1[:, b, :], cst[f'W1k{k}'],
                                     x_pad[0:P, b, k:k + T],
                                     start=(k == 0), stop=(k == 2))
            g1c = tmp.tile([32, BL, T], F32, name="g1c", tag="cnn_tmp")
            nc.scalar.activation(g1c, ps_c1, AF.Gelu,
                                 scale=cst['cnn1s'][:, 0:1], bias=cst['cnn1b'][:, 0:1])
        if dbg:
            dbg('dbg_g1c', g1c)
        pool_pad = tmp.tile([32, BL, 258], F32, tag="pool_pad")
        nc.vector.memset(pool_pad[:, :, 0:1], 0.0)
        nc.vector.memset(pool_pad[:, :, 257:258], 0.0)
        for b in range(BL):
            gv = g1c[:, b, :].rearrange("p (l two) -> p l two", two=2)
            nc.vector.tensor_tensor(pool_pad[:, b, 1:257], gv[:, :, 0],
                                    gv[:, :, 1], op=OP.max)
        if dbg:
            dbg('dbg_pool', pool_pad)
        with tc.tile_pool(name="ps_cnn2", bufs=2, space="PSUM") as psp:
            g2c = tmp.tile([64, BL, 256], F32, tag="cnn_g2c")
            for b in range(BL):
                ps_c2 = psp.tile([64, 256], F32, name="ps_c2", tag="ps_c2")
                for k in range(3):
                    nc.tensor.matmul(ps_c2, cst[f'W2k{k}'],
                                     pool_pad[:, b, k:k + 256],
                                     start=(k == 0), stop=(k == 2))
                nc.scalar.activation(g2c[:, b, :], ps_c2, AF.Gelu,
                                     scale=cst['cnn2s'][:, 0:1],
                                     bias=cst['cnn2b'][:, 0:1])
        if dbg:
            dbg('dbg_g2c', g2c)
        for b in range(BL):
            nc.vector.tensor_reduce(f_inB[0:64, b:b + 1], g2c[:, b, :],
                                    axis=mybir.AxisListType.X, op=OP.add)

        # ---- physics ----
        phys_sb = big.tile([5, BL], F32, tag="phys_sb")
        nc.sync.dma_start(phys_sb, dram['phys_loc'].ap().rearrange("b p -> p b"))
        with tc.tile_pool(name="ps_ph", bufs=1, space="PSUM") as psp:
            ps_p1 = psp.tile([16, BL], F32, tag="ps_p1")
            nc.tensor.matmul(ps_p1, cst['Wp1T'], phys_sb, start=True, stop=True)
            ph1 = tmp.tile([16, BL], F32, tag="ph1")
            nc.scalar.activation(ph1, ps_p1, AF.Gelu, bias=cst['bp1'][:, 0:1])
            ps_p2 = psp.tile([32, BL], F32, tag="ps_p2")
            nc.tensor.matmul(ps_p2, cst['Wp2T'], ph1, start=True, stop=True)
            nc.scalar.activation(f_inB[64:96, :], ps_p2, AF.Gelu,
                                 bias=cst['bp2'][:, 0:1])

        # ---- fusion + classifier ----
        f_sb = big.tile([128, BL], F32, tag="f_sb")
        with tc.tile_pool(name="ps_hd", bufs=1, space="PSUM") as psp:
            gf1 = tmp.tile([128, 2, BL], F32, tag="gf1")
            for half in range(2):
                ps_z = psp.tile([128, BL], F32, tag=f"ps_z1{half}")
                nc.tensor.matmul(ps_z, cst['Wf1Ta'][:, half * 128:(half + 1) * 128],
                                 f_inA, start=True, stop=False)
                nc.tensor.matmul(ps_z, cst['Wf1Tb'][:, half * 128:(half + 1) * 128],
                                 f_inB, start=False, stop=True)
                nc.scalar.activation(gf1[:, half, :], ps_z, AF.Gelu,
                                     scale=cst['f1s'][:, half:half + 1],
                                     bias=cst['f1b'][:, half:half + 1])
            ps_z2 = psp.tile([128, BL], F32, tag="ps_z2")
            nc.tensor.matmul(ps_z2, cst['Wf2Ta'], gf1[:, 0, :], start=True, stop=False)
            nc.tensor.matmul(ps_z2, cst['Wf2Tb'], gf1[:, 1, :], start=False, stop=True)
            nc.scalar.activation(f_sb, ps_z2, AF.Gelu,
                                 scale=cst['f2s'][:, 0:1], bias=cst['f2b'][:, 0:1])
            ps_h1 = psp.tile([64, BL], F32, tag="ps_h1")
            nc.tensor.matmul(ps_h1, cst['Wc1T'], f_sb, start=True, stop=True)
            h1 = tmp.tile([65, BL], F32, tag="h1")
            nc.vector.memset(h1[64:65, :], 1.0)
            nc.scalar.activation(h1[0:64, :], ps_h1, AF.Gelu, bias=cst['bc1'][:, 0:1])
            ps_lg = psp.tile([NUM_CLASSES, BL], F32, tag="ps_lg")
            nc.tensor.matmul(ps_lg, cst['Wc2T'], h1, start=True, stop=True)
            lg = tmp.tile([NUM_CLASSES, BL], F32, tag="lg")
            nc.scalar.activation(lg, ps_lg, AF.Copy)

        if dbg:
            dbg('dbg_fA', f_inA)
            dbg('dbg_fB', f_inB)
        nc.sync.dma_start(out_logits.ap().rearrange("b c -> c b"), lg)
        nc.sync.dma_start(out_f.ap().rearrange("b c -> c b"), f_sb)

    if split_waits:
        _split_sync_waits(nc)
    return nc


_BUILT = None


def _get_module():
    global _BUILT
    if _BUILT is None:
        _BUILT = build_module()
    return _BUILT


def kernel(x, physics_features, params):
    x = np.ascontiguousarray(x, np.float32)
    phys = np.ascontiguousarray(physics_features, np.float32)
    c = _host_constants(params)

    nc = _get_module()
    cpack = _pack_consts(c)
    in_maps = []
    for core in range(NCORES):
        sl = slice(core * BL, (core + 1) * BL)
        m = {'x_loc': x[sl], 'phys_loc': phys[sl],
             'xones': c['xones'], 'cpack': cpack}
        in_maps.append(m)
    global LAST_RESULT
    res = run_bass_kernel_spmd(nc, in_maps, core_ids=list(range(NCORES)),
                               trace=TRACE)
    LAST_RESULT = res
    logits = np.concatenate([r['logits'] for r in res.results], 0)
    f = np.concatenate([r['f'] for r in res.results], 0)
    return logits, f
